# revision 11
# baseline (speedup 1.0000x reference)
"""Trainium2 Bass kernel for nn_DataEmbedding (linear embed + positional + GCN).

out[b,n,t,:] = x[b,n,t,:] @ W_lin + b_lin + pe[t,:] + gcn(emb_table)[n,:]

Sharding: graph-partitioned by destination node. Core k owns nodes
[625k, 625(k+1)) and produces the output shard out[:, 625k:625(k+1), :, :].
No collectives. Host does index/layout prep only (edge sort, dense-adjacency
packing, padding); the tensor math runs on device.

v2 design (output-bandwidth roofline):
- Output written in bf16 from 128-partition SBUF tiles via sync-HWDGE: each
  per-block store is one contiguous DRAM range split evenly over all 16 SDMA
  engines (125-partition stores degrade to 5 engines - measured).
- GCN as dense matmul: host packs edges into A'[src 5120, dst 640] bf16
  (scipy-COO-style duplicate coalescing + self loops); device computes
  Y^T = sum_j emb'_j^T @ A'_j with emb' = dinv[src]*emb (bf16), then
  ve = dinv[dst] * (Y @ W_gcn) + b_gcn. No gathers, no edge scatter.
- Main linear as bf16 matmuls (fp32 moving operand streams at half rate);
  PSUM + ve adds are split between Vector and GpSimd engines, writing bf16.
"""

import numpy as np
import ml_dtypes

import concourse.bacc as bacc
import concourse.bass as bass
import concourse.mybir as mybir
from concourse.bass_utils import run_bass_kernel_spmd
from concourse.tile import TileContext

# problem constants (hardcoded per contract)
B, N, T, CIN, D, E = 8, 5000, 12, 3, 256, 160000
NCORES = 8
NPC = N // NCORES        # real nodes per core = 625
BLK = 128                # nodes per block / output partition tile
NBLK = 5                 # blocks per core; NBLK*BLK = 640 (15 pad rows)
NPAD = NBLK * BLK        # padded local nodes = 640
NT = (N + 127) // 128    # global 128-node src tiles = 40
NG = NT * 128            # padded global nodes = 5120
KX = 3 * T + 2           # main matmul contraction: (t,c) rows + pe + bias = 38
TD = T * D               # 3072

f32 = mybir.dt.float32
b16 = mybir.dt.bfloat16

_KERNEL_CACHE: dict = {}


def _pe_table() -> np.ndarray:
    pos = np.arange(T, dtype=np.float32)[:, None]
    div = np.exp(np.arange(0, D, 2, dtype=np.float32) * (-np.log(10000.0) / D))
    pe = np.zeros((T, D), dtype=np.float32)
    pe[:, 0::2] = np.sin(pos * div)
    pe[:, 1::2] = np.cos(pos * div)
    return pe


def _prep(x, edge_index, weights, W_lin, b_lin):
    """Host-side sharding/layout prep: edge -> dense adjacency packing,
    padding, matmul operand layout. (COO->dense with standard duplicate
    coalescing; plus self-loops, as in PyG GCNConv.)"""
    ei = np.asarray(edge_index).astype(np.int64)
    w = np.asarray(weights, dtype=np.float32)
    row2 = np.concatenate([ei[0], np.arange(N, dtype=np.int64)])
    col2 = np.concatenate([ei[1], np.arange(N, dtype=np.int64)])
    w2 = np.concatenate([w, np.ones(N, dtype=np.float32)])

    # padded per-node incoming-weight lists for on-device degree = row-sum
    cnt = np.bincount(col2, minlength=N)
    L = int(max(8, ((cnt.max() + 7) // 8) * 8))
    order = np.argsort(col2, kind="stable")
    col_s, w_s = col2[order], w2[order]
    starts = np.searchsorted(col_s, np.arange(N)).astype(np.int64)
    offs = np.arange(len(col_s), dtype=np.int64) - starts[col_s]
    wpad = np.zeros((NG, L), dtype=np.float32)
    wpad[col_s, offs] = w_s
    wpad[N:, 0] = 1.0  # pad rows: deg=1 so dinv stays finite
    # SBUF-native [128, NT*L]: partition p, col j*L+l  <-> global node 128j+p
    wpad_pm = np.ascontiguousarray(
        wpad.reshape(NT, 128, L).transpose(1, 0, 2).reshape(128, NT * L)
    ).astype(ml_dtypes.bfloat16)

    # main-matmul rhs [KX, T*D]: rows 3t+c carry W_lin[c] in the t-block of
    # columns (block "diagonal"), row 36 = positional encoding, row 37 = b_lin
    pe = _pe_table()
    rhs38 = np.zeros((KX, TD), dtype=np.float32)
    Wl = np.asarray(W_lin, np.float32)
    for t in range(T):
        for c in range(CIN):
            rhs38[3 * t + c, t * D: (t + 1) * D] = Wl[c]
    rhs38[36] = pe.reshape(-1)
    rhs38[37] = np.tile(np.asarray(b_lin, dtype=np.float32), T)

    xa = np.asarray(x, dtype=np.float32)
    per_core = []
    for k in range(NCORES):
        lo = k * NPC
        # dense normalized-adjacency input A'[src, dst_local] (bf16), tiled
        # [128, NT*NPAD]: partition p, col j*NPAD+q <-> A'[128j+p, q]
        m = (col2 >= lo) & (col2 < lo + NPC)
        A = np.zeros((NG, NPAD), dtype=np.float32)
        np.add.at(A, (row2[m], col2[m] - lo), w2[m])
        A_pm = np.ascontiguousarray(
            A.reshape(NT, 128, NPAD).transpose(1, 0, 2).reshape(128, NT * NPAD)
        ).astype(ml_dtypes.bfloat16)

        # local-dest degree lists [128, NBLK*L]: partition q%128, blk q//128
        wloc = np.zeros((NPAD, L), dtype=np.float32)
        wloc[:NPC] = wpad[lo: lo + NPC]
        wloc[NPC:, 0] = 1.0
        wloc_pm = np.ascontiguousarray(
            wloc.reshape(NBLK, BLK, L).transpose(1, 0, 2).reshape(BLK, NBLK * L)
        ).astype(ml_dtypes.bfloat16)

        # x in matmul-ready lhsT layout [NBLK, KX, B*BLK]; K rows are (t,c)
        # pairs then two ones-rows (pe, bias); batches along free dim
        xs = np.zeros((B, NPAD, T, CIN), dtype=np.float32)
        xs[:, :NPC] = xa[:, lo: lo + NPC]
        xs = xs.reshape(B, NBLK, BLK, T, CIN)
        x38 = np.ones((NBLK, KX, B, BLK), dtype=np.float32)
        x38[:, : 3 * T] = xs.transpose(1, 3, 4, 0, 2).reshape(NBLK, 3 * T, B, BLK)
        per_core.append(
            {
                "Apm": A_pm,
                "wpad_loc": wloc_pm,
                "x38": np.ascontiguousarray(
                    x38.reshape(NBLK, KX, B * BLK).transpose(1, 0, 2)
                    .reshape(KX, NBLK * B * BLK)).astype(ml_dtypes.bfloat16),
            }
        )
    return per_core, wpad_pm, rhs38.astype(ml_dtypes.bfloat16), L


def _build_kernel(L: int):
    key = L
    if key in _KERNEL_CACHE:
        return _KERNEL_CACHE[key]

    nc = bacc.Bacc(num_swdge_queues=4)
    x38_d = nc.declare_dram_parameter("x38", [KX, NBLK * B * BLK], b16, isOutput=False)
    A_d = nc.declare_dram_parameter("Apm", [128, NT * NPAD], b16, isOutput=False)
    wpad_d = nc.declare_dram_parameter("wpad", [128, NT * L], b16, isOutput=False)
    wloc_d = nc.declare_dram_parameter("wpad_loc", [BLK, NBLK * L], b16, isOutput=False)
    emb_d = nc.declare_dram_parameter("emb_pad", [NG, D], b16, isOutput=False)
    wg_d = nc.declare_dram_parameter("W_gcn", [128, 2 * D], b16, isOutput=False)
    bg_d = nc.declare_dram_parameter("b_gcn", [1, D], f32, isOutput=False)
    rhs38_d = nc.declare_dram_parameter("rhs38", [KX, TD], b16, isOutput=False)
    # device-chosen layout; host reassembles [B, 625, T, D] from [blk, p, b, td]
    out_d = nc.declare_dram_parameter("out", [NBLK, BLK, B, TD], b16, isOutput=True)

    with TileContext(nc) as tc:
        with tc.tile_pool(name="keep", bufs=1) as kp:
            # persistent small tiles
            rhs38b = kp.tile([KX, TD], b16)
            wgb = kp.tile([128, 2 * D], b16)     # W_gcn halves, bf16
            yT = kp.tile([128, 2 * NPAD], b16)   # Y^T halves: [d_half, dst]
            xall = kp.tile([KX, NBLK * B * BLK], b16)
            ve_all = kp.tile([BLK, NBLK * D], f32)
            b_rep = kp.tile([BLK, D], f32)
            deg_cat = kp.tile([128, NT + NBLK], f32)
            rec_cat = kp.tile([128, NT + NBLK], f32)
            dinv_cat = kp.tile([128, NT + NBLK], f32)
            ones_row = kp.tile([1, BLK], f32)
            nc.vector.memset(ones_row[:], 1.0)
            bg_row = kp.tile([1, D], f32)

            with (
                tc.tile_pool(name="gcn", bufs=1) as gp,
                tc.tile_pool(name="yp", bufs=1, space="PSUM") as ypp,
            ):
                # small/critical loads ride the scalar HWDGE ring; the bulk
                # emb/A' chunks ride sync (stores only start much later)
                w_all = gp.tile([128, NT * L], b16)
                wl_all = gp.tile([BLK, NBLK * L], b16)
                nc.scalar.dma_start(out=w_all[:], in_=wpad_d[:])
                nc.scalar.dma_start(out=wl_all[:], in_=wloc_d[:])
                nc.scalar.dma_start(out=xall[:], in_=x38_d[:])
                nc.scalar.dma_start(out=rhs38b[:], in_=rhs38_d[:])
                nc.scalar.dma_start(out=wgb[:], in_=wg_d[:])
                nc.scalar.dma_start(out=bg_row[:], in_=bg_d[:])
                A_s = gp.tile([128, NT * NPAD], b16)
                embf = gp.tile([128, NT * D], b16)
                NCH = 8
                CJ = NT // NCH
                for c in range(NCH):
                    j0, j1 = c * CJ, (c + 1) * CJ
                    nc.sync.dma_start(
                        out=embf[:, j0 * D: j1 * D].rearrange(
                            "p (a d) -> p a d", d=D),
                        in_=emb_d[j0 * 128: j1 * 128].rearrange(
                            "(a p) d -> p a d", p=128),
                    )
                    nc.sync.dma_start(
                        out=A_s[:, j0 * NPAD: j1 * NPAD],
                        in_=A_d[:, j0 * NPAD: j1 * NPAD],
                    )
                embs = gp.tile([128, NT * D], b16)

                # ---- degrees -> dinv (batched reciprocal+sqrt) ----
                for j in range(NT):
                    nc.vector.reduce_sum(
                        out=deg_cat[:, j: j + 1], in_=w_all[:, j * L:(j + 1) * L],
                        axis=mybir.AxisListType.X,
                    )
                for blk in range(NBLK):
                    nc.vector.reduce_sum(
                        out=deg_cat[:, NT + blk: NT + blk + 1],
                        in_=wl_all[:, blk * L:(blk + 1) * L],
                        axis=mybir.AxisListType.X,
                    )
                nc.vector.reciprocal(rec_cat[:], deg_cat[:])
                nc.scalar.sqrt(dinv_cat[:], rec_cat[:])

                # ---- emb' = dinv[src] * emb, cast bf16 ----
                for j in range(NT):
                    nc.vector.tensor_scalar_mul(
                        embs[:, j * D:(j + 1) * D],
                        embf[:, j * D:(j + 1) * D],
                        dinv_cat[:, j: j + 1],
                    )

                # ---- Y^T[dh] = sum_j emb'_j[:,dh]^T @ A'_j  (PSUM accum) ----
                ypA = [ypp.tile([128, 512], f32, space="PSUM", tag=f"ypA{dh}",
                                name=f"ypA{dh}") for dh in range(2)]
                ypB = [ypp.tile([128, NPAD - 512], f32, space="PSUM", tag=f"ypB{dh}",
                                name=f"ypB{dh}") for dh in range(2)]
                for j in range(NT):
                    for dh in range(2):
                        lhsT = embs[:, j * D + dh * 128: j * D + (dh + 1) * 128]
                        a0 = j * NPAD
                        nc.tensor.matmul(
                            ypA[dh][:], lhsT=lhsT, rhs=A_s[:, a0: a0 + 512],
                            start=(j == 0), stop=(j == NT - 1),
                        )
                        nc.tensor.matmul(
                            ypB[dh][:], lhsT=lhsT, rhs=A_s[:, a0 + 512: a0 + NPAD],
                            start=(j == 0), stop=(j == NT - 1),
                        )
                for dh in range(2):
                    nc.vector.tensor_copy(yT[:, dh * NPAD: dh * NPAD + 512], ypA[dh][:])
                    nc.vector.tensor_copy(
                        yT[:, dh * NPAD + 512: (dh + 1) * NPAD], ypB[dh][:]
                    )

            # ---- region 2: ve matmuls flow straight into the main loop so
            # the PE stream stays dense (HAM stays warm) ----
            with (
                tc.tile_pool(name="vpbr", bufs=1, space="PSUM") as vpbr,
                tc.tile_pool(name="mps", bufs=2, space="PSUM") as mps,
                tc.tile_pool(name="xload", bufs=2) as xload,
                tc.tile_pool(name="stgp", bufs=3) as stgp,
                tc.tile_pool(name="outp", bufs=2) as outp,
            ):
                # b_rep = ones(128,1) @ b_gcn(1,256)
                br = vpbr.tile([BLK, D], f32, space="PSUM", tag="vp")
                nc.tensor.matmul(br[:], lhsT=ones_row[0:1, :], rhs=bg_row[0:1, :],
                                 start=True, stop=True)
                nc.vector.tensor_copy(b_rep[:], br[:])

                # ---- ve = dinv[dst] * (Y @ W_gcn) + b_gcn ----
                for blk in range(NBLK):
                    vp = vpbr.tile([BLK, D], f32, space="PSUM", tag="vp")
                    for dh in range(2):
                        nc.tensor.matmul(
                            vp[:],
                            lhsT=yT[:, dh * NPAD + blk * BLK:
                                    dh * NPAD + (blk + 1) * BLK],
                            rhs=wgb[:, dh * D:(dh + 1) * D],
                            start=(dh == 0), stop=(dh == 1),
                        )
                    ve = ve_all[:, blk * D:(blk + 1) * D]
                    nc.vector.tensor_scalar_mul(ve, vp[:], dinv_cat[:, NT + blk: NT + blk + 1])
                    nc.vector.tensor_add(ve, ve, b_rep[:])

                # ---- main loop: per block, linear matmuls + ve add + store ----
                for blk in range(NBLK):
                    ve3 = (
                        ve_all[:, blk * D:(blk + 1) * D]
                        .rearrange("p d -> p () d")
                        .to_broadcast([BLK, T // 2, D])
                    )
                    # bf16 ve tile (half-T): dense in1 for the adds (2x mode)
                    vebt = xload.tile([BLK, 6 * D], b16, tag="vebt")
                    nc.vector.tensor_copy(
                        vebt[:].rearrange("p (t d) -> p t d", d=D), ve3)
                    osb = outp.tile([BLK, B * TD], b16, tag="osb")
                    for b in range(B):
                        lhsT = xall[:, (blk * B + b) * BLK:(blk * B + b + 1) * BLK]
                        for half in range(2):
                            mp = mps.tile([BLK, 3 * 512], f32, space="PSUM", tag="mp")
                            for i in range(3):
                                tp = half * 3 + i  # t-pair index
                                nc.tensor.matmul(
                                    mp[:, i * 512:(i + 1) * 512],
                                    lhsT=lhsT,
                                    rhs=rhs38b[:, tp * 512:(tp + 1) * 512],
                                    start=True, stop=True,
                                )
                            # warm-keeper: PE idle gaps re-throttle the HAM
                            # clock gate; a consumer-less matmul per group
                            # keeps the stream dense so MMs run at 2.4 GHz
                            dum = vpbr.tile([BLK, D], f32, space="PSUM", tag="vp")
                            nc.tensor.matmul(
                                dum[:], lhsT=lhsT, rhs=rhs38b[:, 0:D],
                                start=True, stop=True,
                            )
                            oseg = osb[:, b * TD + half * 1536:
                                       b * TD + (half + 1) * 1536]
                            if (b * 2 + half) < 5:
                                nc.vector.tensor_tensor(
                                    out=oseg, in0=mp[:], in1=vebt[:],
                                    op=mybir.AluOpType.add,
                                )
                            else:
                                stg = stgp.tile([BLK, 1536], b16, tag="stg")
                                nc.scalar.copy(stg[:], mp[:])
                                nc.vector.tensor_tensor(
                                    out=oseg, in0=stg[:], in1=vebt[:],
                                    op=mybir.AluOpType.add,
                                )
                        if b == 3:
                            nc.sync.dma_start(
                                out=out_d[blk, :, 0: B // 2],
                                in_=osb[:, : B // 2 * TD].rearrange(
                                    "p (b c) -> p b c", b=B // 2),
                            )
                    nc.sync.dma_start(
                        out=out_d[blk, :, B // 2:],
                        in_=osb[:, B // 2 * TD:].rearrange(
                            "p (b c) -> p b c", b=B // 2),
                    )

    nc.finalize()
    _KERNEL_CACHE[key] = nc
    return nc


LAST_RESULTS = None  # BassKernelResults of the most recent run (for profiling)


def kernel(x, x_mark, edge_index, weights, W_lin, b_lin, emb_table, W_gcn, b_gcn):
    global LAST_RESULTS
    per_core, wpad_pm, rhs38, L = _prep(x, edge_index, weights, W_lin, b_lin)
    nc = _build_kernel(L)
    emb_pad = np.zeros((NG, D), dtype=ml_dtypes.bfloat16)
    emb_pad[:N] = np.asarray(emb_table, dtype=np.float32).astype(ml_dtypes.bfloat16)
    shared = {
        "wpad": wpad_pm,
        "emb_pad": emb_pad,
        "W_gcn": np.ascontiguousarray(
            np.asarray(W_gcn, dtype=np.float32).reshape(2, 128, D)
            .transpose(1, 0, 2).reshape(128, 2 * D)).astype(ml_dtypes.bfloat16),
        "b_gcn": np.asarray(b_gcn, dtype=np.float32).reshape(1, D),
        "rhs38": rhs38,
    }
    in_maps = [{**shared, **pc} for pc in per_core]
    res = run_bass_kernel_spmd(nc, in_maps, list(range(NCORES)))
    LAST_RESULTS = res
    shards = []
    for k in range(NCORES):
        o = np.asarray(res.results[k]["out"]).astype(np.float32)
        # [NBLK, BLK, B, TD] -> [B, NPAD, T, D] -> drop pad rows
        o = o.reshape(NBLK * BLK, B, T, D).transpose(1, 0, 2, 3)[:, :NPC]
        shards.append(o)
    return np.concatenate(shards, axis=1)


# revision 12
# speedup vs baseline: 1.1971x; 1.1971x over previous
"""Trainium2 Bass kernel for nn_DataEmbedding (linear embed + positional + GCN).

out[b,n,t,:] = x[b,n,t,:] @ W_lin + b_lin + pe[t,:] + gcn(emb_table)[n,:]

Sharding: graph-partitioned by destination node. Core k owns nodes
[625k, 625(k+1)) and produces the output shard out[:, 625k:625(k+1), :, :].
No collectives. Host does index/layout prep only (edge sort, dense-adjacency
packing, padding); the tensor math runs on device.

v2 design (output-bandwidth roofline):
- Output written in bf16 from 128-partition SBUF tiles via sync-HWDGE: each
  per-block store is one contiguous DRAM range split evenly over all 16 SDMA
  engines (125-partition stores degrade to 5 engines - measured).
- GCN as dense matmul: host packs edges into A'[src 5120, dst 640] bf16
  (scipy-COO-style duplicate coalescing + self loops); device computes
  Y^T = sum_j emb'_j^T @ A'_j with emb' = dinv[src]*emb (bf16), then
  ve = dinv[dst] * (Y @ W_gcn) + b_gcn. No gathers, no edge scatter.
- Main linear as bf16 matmuls (fp32 moving operand streams at half rate);
  PSUM + ve adds are split between Vector and GpSimd engines, writing bf16.
"""

import numpy as np
import ml_dtypes

import concourse.bacc as bacc
import concourse.bass as bass
import concourse.mybir as mybir
from concourse.bass_utils import run_bass_kernel_spmd
from concourse.tile import TileContext

# problem constants (hardcoded per contract)
B, N, T, CIN, D, E = 8, 5000, 12, 3, 256, 160000
NCORES = 8
NPC = N // NCORES        # real nodes per core = 625
BLK = 128                # nodes per block / output partition tile
NBLK = 5                 # blocks per core; NBLK*BLK = 640 (15 pad rows)
NPAD = NBLK * BLK        # padded local nodes = 640
NT = (N + 127) // 128    # global 128-node src tiles = 40
NG = NT * 128            # padded global nodes = 5120
KX = 3 * T + 2           # main matmul contraction: (t,c) rows + pe + bias = 38
TD = T * D               # 3072

f32 = mybir.dt.float32
b16 = mybir.dt.bfloat16

_KERNEL_CACHE: dict = {}


def _pe_table() -> np.ndarray:
    pos = np.arange(T, dtype=np.float32)[:, None]
    div = np.exp(np.arange(0, D, 2, dtype=np.float32) * (-np.log(10000.0) / D))
    pe = np.zeros((T, D), dtype=np.float32)
    pe[:, 0::2] = np.sin(pos * div)
    pe[:, 1::2] = np.cos(pos * div)
    return pe


def _prep(x, edge_index, weights, W_lin, b_lin):
    """Host-side sharding/layout prep: edge -> dense adjacency packing,
    padding, matmul operand layout. (COO->dense with standard duplicate
    coalescing; plus self-loops, as in PyG GCNConv.)"""
    ei = np.asarray(edge_index).astype(np.int64)
    w = np.asarray(weights, dtype=np.float32)
    row2 = np.concatenate([ei[0], np.arange(N, dtype=np.int64)])
    col2 = np.concatenate([ei[1], np.arange(N, dtype=np.int64)])
    w2 = np.concatenate([w, np.ones(N, dtype=np.float32)])

    # padded per-node incoming-weight lists for on-device degree = row-sum
    cnt = np.bincount(col2, minlength=N)
    L = int(max(8, ((cnt.max() + 7) // 8) * 8))
    order = np.argsort(col2, kind="stable")
    col_s, w_s = col2[order], w2[order]
    starts = np.searchsorted(col_s, np.arange(N)).astype(np.int64)
    offs = np.arange(len(col_s), dtype=np.int64) - starts[col_s]
    wpad = np.zeros((NG, L), dtype=np.float32)
    wpad[col_s, offs] = w_s
    wpad[N:, 0] = 1.0  # pad rows: deg=1 so dinv stays finite
    # SBUF-native [128, NT*L]: partition p, col j*L+l  <-> global node 128j+p
    wpad_pm = np.ascontiguousarray(
        wpad.reshape(NT, 128, L).transpose(1, 0, 2).reshape(128, NT * L)
    ).astype(ml_dtypes.bfloat16)

    # main-matmul rhs [KX, T*D]: rows 3t+c carry W_lin[c] in the t-block of
    # columns (block "diagonal"), row 36 = positional encoding, row 37 = b_lin
    pe = _pe_table()
    rhs38 = np.zeros((KX, TD), dtype=np.float32)
    Wl = np.asarray(W_lin, np.float32)
    for t in range(T):
        for c in range(CIN):
            rhs38[3 * t + c, t * D: (t + 1) * D] = Wl[c]
    rhs38[36] = pe.reshape(-1)
    rhs38[37] = np.tile(np.asarray(b_lin, dtype=np.float32), T)

    xa = np.asarray(x, dtype=np.float32)
    per_core = []
    for k in range(NCORES):
        lo = k * NPC
        # dense normalized-adjacency input A'[src, dst_local] (bf16), tiled
        # [128, NT*NPAD]: partition p, col j*NPAD+q <-> A'[128j+p, q]
        m = (col2 >= lo) & (col2 < lo + NPC)
        A = np.zeros((NG, NPAD), dtype=np.float32)
        np.add.at(A, (row2[m], col2[m] - lo), w2[m])
        A_pm = np.ascontiguousarray(
            A.reshape(NT, 128, NPAD).transpose(1, 0, 2).reshape(128, NT * NPAD)
        ).astype(ml_dtypes.bfloat16)

        # local-dest degree lists [128, NBLK*L]: partition q%128, blk q//128
        wloc = np.zeros((NPAD, L), dtype=np.float32)
        wloc[:NPC] = wpad[lo: lo + NPC]
        wloc[NPC:, 0] = 1.0
        wloc_pm = np.ascontiguousarray(
            wloc.reshape(NBLK, BLK, L).transpose(1, 0, 2).reshape(BLK, NBLK * L)
        ).astype(ml_dtypes.bfloat16)

        # x in matmul-ready lhsT layout [NBLK, KX, B*BLK]; K rows are (t,c)
        # pairs then two ones-rows (pe, bias); batches along free dim
        xs = np.zeros((B, NPAD, T, CIN), dtype=np.float32)
        xs[:, :NPC] = xa[:, lo: lo + NPC]
        xs = xs.reshape(B, NBLK, BLK, T, CIN)
        x38 = np.ones((NBLK, KX, B, BLK), dtype=np.float32)
        x38[:, : 3 * T] = xs.transpose(1, 3, 4, 0, 2).reshape(NBLK, 3 * T, B, BLK)
        per_core.append(
            {
                "Apm": A_pm,
                "wpad_loc": wloc_pm,
                "x38": np.ascontiguousarray(
                    x38.reshape(NBLK, KX, B * BLK).transpose(1, 0, 2)
                    .reshape(KX, NBLK * B * BLK)).astype(ml_dtypes.bfloat16),
            }
        )
    return per_core, wpad_pm, rhs38.astype(ml_dtypes.bfloat16), L


def _build_kernel(L: int):
    key = L
    if key in _KERNEL_CACHE:
        return _KERNEL_CACHE[key]

    nc = bacc.Bacc(num_swdge_queues=4)
    x38_d = nc.declare_dram_parameter("x38", [KX, NBLK * B * BLK], b16, isOutput=False)
    A_d = nc.declare_dram_parameter("Apm", [128, NT * NPAD], b16, isOutput=False)
    wpad_d = nc.declare_dram_parameter("wpad", [128, NT * L], b16, isOutput=False)
    wloc_d = nc.declare_dram_parameter("wpad_loc", [BLK, NBLK * L], b16, isOutput=False)
    emb_d = nc.declare_dram_parameter("emb_pad", [NG, D], b16, isOutput=False)
    wg_d = nc.declare_dram_parameter("W_gcn", [128, 2 * D], b16, isOutput=False)
    bg_d = nc.declare_dram_parameter("b_gcn", [1, D], f32, isOutput=False)
    rhs38_d = nc.declare_dram_parameter("rhs38", [KX, TD], b16, isOutput=False)
    # device-chosen layout; host reassembles [B, 625, T, D] from [blk, p, b, td]
    out_d = nc.declare_dram_parameter("out", [NBLK, BLK, B, TD], b16, isOutput=True)

    with TileContext(nc) as tc:
        with tc.tile_pool(name="keep", bufs=1) as kp:
            # persistent small tiles
            rhs38b = kp.tile([KX, TD], b16)
            wgb = kp.tile([128, 2 * D], b16)     # W_gcn halves, bf16
            yT = kp.tile([128, 2 * NPAD], b16)   # Y^T halves: [d_half, dst]
            xall = kp.tile([KX, NBLK * B * BLK], b16)
            ve_all = kp.tile([BLK, NBLK * D], f32)
            b_rep = kp.tile([BLK, D], f32)
            deg_cat = kp.tile([128, NT + NBLK], f32)
            rec_cat = kp.tile([128, NT + NBLK], f32)
            dinv_cat = kp.tile([128, NT + NBLK], f32)
            ones_row = kp.tile([1, BLK], f32)
            nc.vector.memset(ones_row[:], 1.0)
            bg_row = kp.tile([1, D], f32)

            with (
                tc.tile_pool(name="gcn", bufs=1) as gp,
                tc.tile_pool(name="yp", bufs=1, space="PSUM") as ypp,
            ):
                # small/critical loads ride the scalar HWDGE ring; the bulk
                # emb/A' chunks ride sync (stores only start much later)
                w_all = gp.tile([128, NT * L], b16)
                wl_all = gp.tile([BLK, NBLK * L], b16)
                nc.scalar.dma_start(out=w_all[:], in_=wpad_d[:])
                nc.scalar.dma_start(out=wl_all[:], in_=wloc_d[:])
                nc.scalar.dma_start(out=xall[:], in_=x38_d[:])
                nc.scalar.dma_start(out=rhs38b[:], in_=rhs38_d[:])
                nc.scalar.dma_start(out=wgb[:], in_=wg_d[:])
                nc.scalar.dma_start(out=bg_row[:], in_=bg_d[:])
                A_s = gp.tile([128, NT * NPAD], b16)
                embf = gp.tile([128, NT * D], b16)
                NCH = 8
                CJ = NT // NCH
                for c in range(NCH):
                    j0, j1 = c * CJ, (c + 1) * CJ
                    nc.sync.dma_start(
                        out=embf[:, j0 * D: j1 * D].rearrange(
                            "p (a d) -> p a d", d=D),
                        in_=emb_d[j0 * 128: j1 * 128].rearrange(
                            "(a p) d -> p a d", p=128),
                    )
                    nc.sync.dma_start(
                        out=A_s[:, j0 * NPAD: j1 * NPAD],
                        in_=A_d[:, j0 * NPAD: j1 * NPAD],
                    )
                embs = gp.tile([128, NT * D], b16)

                # ---- degrees -> dinv (batched reciprocal+sqrt) ----
                for j in range(NT):
                    nc.vector.reduce_sum(
                        out=deg_cat[:, j: j + 1], in_=w_all[:, j * L:(j + 1) * L],
                        axis=mybir.AxisListType.X,
                    )
                for blk in range(NBLK):
                    nc.vector.reduce_sum(
                        out=deg_cat[:, NT + blk: NT + blk + 1],
                        in_=wl_all[:, blk * L:(blk + 1) * L],
                        axis=mybir.AxisListType.X,
                    )
                nc.vector.reciprocal(rec_cat[:], deg_cat[:])
                nc.scalar.sqrt(dinv_cat[:], rec_cat[:])

                # ---- emb' = dinv[src] * emb, cast bf16 ----
                for j in range(NT):
                    nc.vector.tensor_scalar_mul(
                        embs[:, j * D:(j + 1) * D],
                        embf[:, j * D:(j + 1) * D],
                        dinv_cat[:, j: j + 1],
                    )

                # ---- Y^T[dh] = sum_j emb'_j[:,dh]^T @ A'_j  (PSUM accum) ----
                ypA = [ypp.tile([128, 512], f32, space="PSUM", tag=f"ypA{dh}",
                                name=f"ypA{dh}") for dh in range(2)]
                ypB = [ypp.tile([128, NPAD - 512], f32, space="PSUM", tag=f"ypB{dh}",
                                name=f"ypB{dh}") for dh in range(2)]
                for j in range(NT):
                    for dh in range(2):
                        lhsT = embs[:, j * D + dh * 128: j * D + (dh + 1) * 128]
                        a0 = j * NPAD
                        nc.tensor.matmul(
                            ypA[dh][:], lhsT=lhsT, rhs=A_s[:, a0: a0 + 512],
                            start=(j == 0), stop=(j == NT - 1),
                        )
                        nc.tensor.matmul(
                            ypB[dh][:], lhsT=lhsT, rhs=A_s[:, a0 + 512: a0 + NPAD],
                            start=(j == 0), stop=(j == NT - 1),
                        )
                for dh in range(2):
                    nc.vector.tensor_copy(yT[:, dh * NPAD: dh * NPAD + 512], ypA[dh][:])
                    nc.vector.tensor_copy(
                        yT[:, dh * NPAD + 512: (dh + 1) * NPAD], ypB[dh][:]
                    )

            # ---- region 2: ve matmuls flow straight into the main loop so
            # the PE stream stays dense (HAM stays warm) ----
            with (
                tc.tile_pool(name="xload", bufs=2) as xload,
                tc.tile_pool(name="stgp", bufs=4) as stgp,
                tc.tile_pool(name="outp", bufs=2) as outp,
            ):
                with tc.tile_pool(name="vpbr", bufs=1, space="PSUM") as vpbr:
                    # b_rep = ones(128,1) @ b_gcn(1,256)
                    br = vpbr.tile([BLK, D], f32, space="PSUM", tag="vp")
                    nc.tensor.matmul(br[:], lhsT=ones_row[0:1, :],
                                     rhs=bg_row[0:1, :], start=True, stop=True)
                    nc.vector.tensor_copy(b_rep[:], br[:])

                    # ---- ve = dinv[dst] * (Y @ W_gcn) + b_gcn ----
                    for blk in range(NBLK):
                        vp = vpbr.tile([BLK, D], f32, space="PSUM", tag="vp")
                        for dh in range(2):
                            nc.tensor.matmul(
                                vp[:],
                                lhsT=yT[:, dh * NPAD + blk * BLK:
                                        dh * NPAD + (blk + 1) * BLK],
                                rhs=wgb[:, dh * D:(dh + 1) * D],
                                start=(dh == 0), stop=(dh == 1),
                            )
                        ve = ve_all[:, blk * D:(blk + 1) * D]
                        nc.vector.tensor_scalar_mul(
                            ve, vp[:], dinv_cat[:, NT + blk: NT + blk + 1])
                        nc.vector.tensor_add(ve, ve, b_rep[:])

                # ---- main loop: per block, linear matmuls + ve add + store ----
                with tc.tile_pool(name="mps", bufs=4, space="PSUM") as mps:
                    for blk in range(NBLK):
                        ve4 = (
                            ve_all[:, blk * D:(blk + 1) * D]
                            .rearrange("p d -> p () d")
                            .to_broadcast([BLK, 4, D])
                        )
                        # bf16 ve tile over a 4-t period: dense in1 for the
                        # adds (the ve pattern repeats every t)
                        vebt = xload.tile([BLK, 4 * D], b16, tag="vebt")
                        nc.vector.tensor_copy(
                            vebt[:].rearrange("p (t d) -> p t d", d=D), ve4)
                        osb = outp.tile([BLK, B * TD], b16, tag="osb")
                        for b in range(B):
                            lhsT = xall[:, (blk * B + b) * BLK:
                                        (blk * B + b + 1) * BLK]
                            for g in range(3):
                                mp = mps.tile([BLK, 1024], f32, space="PSUM",
                                              tag="mp")
                                for i in range(2):
                                    tp = g * 2 + i  # t-pair index
                                    nc.tensor.matmul(
                                        mp[:, i * 512:(i + 1) * 512],
                                        lhsT=lhsT,
                                        rhs=rhs38b[:, tp * 512:(tp + 1) * 512],
                                        start=True, stop=True,
                                    )
                                oseg = osb[:, b * TD + g * 1024:
                                           b * TD + (g + 1) * 1024]
                                if (b * 3 + g) % 3 == 0:
                                    nc.vector.tensor_tensor(
                                        out=oseg, in0=mp[:], in1=vebt[:],
                                        op=mybir.AluOpType.add,
                                    )
                                else:
                                    stg = stgp.tile([BLK, 1024], b16, tag="stg")
                                    nc.scalar.copy(stg[:], mp[:])
                                    nc.vector.tensor_tensor(
                                        out=oseg, in0=stg[:], in1=vebt[:],
                                        op=mybir.AluOpType.add,
                                    )
                            if b == 3:
                                nc.sync.dma_start(
                                    out=out_d[blk, :, 0: B // 2],
                                    in_=osb[:, : B // 2 * TD].rearrange(
                                        "p (b c) -> p b c", b=B // 2),
                                )
                        nc.sync.dma_start(
                            out=out_d[blk, :, B // 2:],
                            in_=osb[:, B // 2 * TD:].rearrange(
                                "p (b c) -> p b c", b=B // 2),
                        )

    nc.finalize()
    _KERNEL_CACHE[key] = nc
    return nc


LAST_RESULTS = None  # BassKernelResults of the most recent run (for profiling)


def kernel(x, x_mark, edge_index, weights, W_lin, b_lin, emb_table, W_gcn, b_gcn):
    global LAST_RESULTS
    per_core, wpad_pm, rhs38, L = _prep(x, edge_index, weights, W_lin, b_lin)
    nc = _build_kernel(L)
    emb_pad = np.zeros((NG, D), dtype=ml_dtypes.bfloat16)
    emb_pad[:N] = np.asarray(emb_table, dtype=np.float32).astype(ml_dtypes.bfloat16)
    shared = {
        "wpad": wpad_pm,
        "emb_pad": emb_pad,
        "W_gcn": np.ascontiguousarray(
            np.asarray(W_gcn, dtype=np.float32).reshape(2, 128, D)
            .transpose(1, 0, 2).reshape(128, 2 * D)).astype(ml_dtypes.bfloat16),
        "b_gcn": np.asarray(b_gcn, dtype=np.float32).reshape(1, D),
        "rhs38": rhs38,
    }
    in_maps = [{**shared, **pc} for pc in per_core]
    res = run_bass_kernel_spmd(nc, in_maps, list(range(NCORES)))
    LAST_RESULTS = res
    shards = []
    for k in range(NCORES):
        o = np.asarray(res.results[k]["out"]).astype(np.float32)
        # [NBLK, BLK, B, TD] -> [B, NPAD, T, D] -> drop pad rows
        o = o.reshape(NBLK * BLK, B, T, D).transpose(1, 0, 2, 3)[:, :NPC]
        shards.append(o)
    return np.concatenate(shards, axis=1)
